# revision 1
# baseline (speedup 1.0000x reference)
import sys

if "/opt/trn_rl_repo" not in sys.path:
    sys.path.insert(0, "/opt/trn_rl_repo")

import numpy as np

NCORES = 8
B = 65536
NPC = B // NCORES    # 8192 images per core
# variable chunk sizes (in 128-image subtiles): small first chunks to get
# compute started early, small last chunks to drain the tail quickly
CHUNKS = [1, 3] + [4] * 14 + [3, 1]
assert sum(CHUNKS) == NPC // 128
AF = 128.0 / 127.5

# conv-as-banded-matmul windows: window w reads input pixels [S[w], S[w]+128)
# and produces output pixels [o0, o0+n).  Output pixels 0..503 go to PSUM
# bank A, 504..575 to bank B (the last window is split across both).
S = [0, 78, 156, 234, 312, 390, 448]
ORANGE = [(0, 103), (103, 78), (181, 78), (259, 78), (337, 78), (415, 78), (493, 83)]
PSPLIT = 512

_cache = {}


def _build():
    from contextlib import ExitStack

    import concourse.tile as tile
    from concourse import bacc, mybir

    f32 = mybir.dt.float32
    f16 = mybir.dt.float16
    Alu = mybir.AluOpType
    Act = mybir.ActivationFunctionType

    nc = bacc.Bacc("TRN2", target_bir_lowering=False, debug=False,
                   num_devices=NCORES)
    x = nc.dram_tensor("x", [NPC, 576], f32, kind="ExternalInput").ap()
    wcv = nc.dram_tensor("wcv", [128, 576], f16, kind="ExternalInput").ap()
    wfc = nc.dram_tensor("wfc", [256, 10], f16, kind="ExternalInput").ap()
    ident = nc.dram_tensor("ident", [128, 128], f16, kind="ExternalInput").ap()
    out = nc.dram_tensor("out", [10, NPC], f32, kind="ExternalOutput").ap()

    with tile.TileContext(nc) as tc, ExitStack() as ctx:
        consts = ctx.enter_context(tc.tile_pool(name="consts", bufs=1))
        wc = consts.tile([128, 576], f16)
        idt = consts.tile([128, 128], f16)
        w1 = consts.tile([128, 10], f16)
        w2 = consts.tile([128, 10], f16)
        nc.sync.dma_start(wc[:], wcv)
        nc.sync.dma_start(idt[:], ident)
        nc.sync.dma_start(w1[:], wfc[0:128, :])
        nc.sync.dma_start(w2[:], wfc[128:256, :])

        xpool = ctx.enter_context(tc.tile_pool(name="xp", bufs=3))
        qpool = ctx.enter_context(tc.tile_pool(name="qp", bufs=3))
        xtpool = ctx.enter_context(tc.tile_pool(name="xtp", bufs=3))
        p0pool = ctx.enter_context(tc.tile_pool(name="p0p", bufs=3))
        p1pool = ctx.enter_context(tc.tile_pool(name="p1p", bufs=3))
        apool = ctx.enter_context(tc.tile_pool(name="apl", bufs=3))
        atpool = ctx.enter_context(tc.tile_pool(name="atp", bufs=2))
        sopool = ctx.enter_context(tc.tile_pool(name="sop", bufs=2))
        pst = ctx.enter_context(tc.tile_pool(name="pst", bufs=2, space="PSUM"))
        pmix = ctx.enter_context(tc.tile_pool(name="pmix", bufs=2, space="PSUM"))
        pca = ctx.enter_context(tc.tile_pool(name="pca", bufs=2, space="PSUM"))

        xv_dram = x.rearrange("(t p) f -> p t f", p=128)

        sb = 0
        for g, n in enumerate(CHUNKS):
            xr = xpool.tile([128, n * 576], f32, tag="xr")
            nc.sync.dma_start(xr[:].rearrange("p (a f) -> p a f", a=n),
                              xv_dram[:, sb:sb + n, :])
            # quantize on DVE: q = RTNE_f16(x*AF + (1536-128))
            q = qpool.tile([128, n * 576], f16, tag="q")
            nc.vector.tensor_scalar(q[:], xr[:], AF, 1536.0 - 128.0,
                                    Alu.mult, Alu.add)

            # transpose to pixel-major via PE identity matmuls; evac with
            # the -1536 bias fold (DVE adds a high-side clamp for free)
            xt = xtpool.tile([128, 7 * n * 128], f16, tag="xt")
            for w in range(7):
                T = pst.tile([128, n * 128], f32, tag="ps", name=f"T{w}")
                for a in range(n):
                    nc.tensor.matmul(T[:, a * 128:(a + 1) * 128],
                                     q[:, a * 576 + S[w]:a * 576 + S[w] + 128],
                                     idt[:], start=True, stop=True)
                xtw = xt[:, w * n * 128:(w + 1) * n * 128]
                if w < 3:
                    nc.vector.tensor_scalar(xtw, T[:], 1536.0, 127.0,
                                            Alu.subtract, Alu.min)
                else:
                    nc.scalar.activation(xtw, T[:], Act.Copy, bias=-1536.0)

            # conv: per subtile, 8 banded matmuls (one per window, last
            # split at the PSUM bank boundary within a 2-bank tile)
            p0 = p0pool.tile([128, n * 576], f16, tag="p0")
            for a in range(n):
                PAB = pca.tile([128, 576], f32)
                for w in range(7):
                    lhs = xt[:, (w * n + a) * 128:(w * n + a + 1) * 128]
                    o0, m = ORANGE[w]
                    if o0 + m <= PSPLIT:
                        nc.tensor.matmul(PAB[:, o0:o0 + m], lhs,
                                         wc[:, o0:o0 + m],
                                         start=True, stop=True)
                    else:
                        nc.tensor.matmul(PAB[:, o0:PSPLIT], lhs,
                                         wc[:, o0:PSPLIT],
                                         start=True, stop=True)
                        nc.tensor.matmul(PAB[:, PSPLIT:o0 + m], lhs,
                                         wc[:, PSPLIT:o0 + m],
                                         start=True, stop=True)
                # relu + evacuate conv result in one bank-spanning op
                if a == 3:
                    nc.vector.tensor_scalar_max(
                        p0[:, a * 576:(a + 1) * 576], PAB[:], 0.0)
                else:
                    nc.scalar.activation(p0[:, a * 576:(a + 1) * 576],
                                         PAB[:], Act.Relu)

            # maxpool stage 1 (row pairs) on DVE, batched
            p1 = p1pool.tile([128, n * 288], f16, tag="p1")
            p0v = p0[:].rearrange("p (s r t c) -> p s r t c", s=n, r=12, t=2)
            p1v = p1[:].rearrange("p (s r c) -> p s r c", s=n, r=12)
            nc.vector.tensor_tensor(p1v, p0v[:, :, :, 0, :],
                                    p0v[:, :, :, 1, :], Alu.max)
            # maxpool stage 2 (col pairs) on DVE
            act = apool.tile([128, n * 144], f16, tag="act")
            p1w = p1[:].rearrange("p (s r c t) -> p s r c t", s=n, r=12, c=12)
            actv = act[:].rearrange("p (s r c) -> p s r c", s=n, r=12)
            nc.vector.tensor_tensor(actv, p1w[:, :, :, :, 0],
                                    p1w[:, :, :, :, 1], Alu.max)
            # +1536 and clip at 1663=127+1536; the f16 write rounds to int.
            # act stays biased by 1536 — the aT evacuations subtract it.
            nc.vector.tensor_scalar(act[:], act[:], 1536.0, 1663.0,
                                    Alu.add, Alu.min)

            # transpose act to feature-major via PE identity matmuls
            pT1 = pmix.tile([128, n * 128], f32, tag="fc")
            pT2 = pmix.tile([128, n * 128], f32, tag="fc")
            for a in range(n):
                nc.tensor.matmul(pT1[:, a * 128:(a + 1) * 128],
                                 act[:, a * 144:a * 144 + 128],
                                 idt[:], start=True, stop=True)
                nc.tensor.matmul(pT2[:, a * 128:(a + 1) * 128],
                                 act[:, a * 144 + 16:a * 144 + 144],
                                 idt[:], start=True, stop=True)
            aT1 = atpool.tile([128, n * 128], f16, tag="aT1")
            aT2 = atpool.tile([128, n * 128], f16, tag="aT2")
            nc.scalar.activation(aT1[:], pT1[:], Act.Copy, bias=-1536.0)
            nc.vector.tensor_scalar_add(aT2[:], pT2[:], -1536.0)

            # FC: out^T[o, b] = sum_k W[k, o] aT[k, b]
            pOT = pmix.tile([10, n * 128], f32, tag="fc")
            nc.tensor.matmul(pOT[:], w1[:], aT1[:], start=True, stop=False)
            nc.tensor.matmul(pOT[:], w2[:], aT2[:], start=False, stop=True)
            so = sopool.tile([10, n * 128], f32, tag="so")
            nc.scalar.copy(so[:], pOT[:])
            nc.sync.dma_start(out[:, sb * 128:(sb + n) * 128], so[:])
            sb += n

    nc.compile()
    return nc


def _prep(conv_w, fc_w):
    cw = np.asarray(conv_w, np.float32).reshape(3, 3)
    wq = (np.round(np.clip(cw, -0.5, 0.5) * 2.0) / 2.0).astype(np.float32)
    fw = np.asarray(fc_w, np.float32)
    wfq = (np.round(np.clip(fw, -0.5, 0.5) * 2.0) / 2.0 / 8.0).astype(np.float32)

    # banded 576x576 conv matrix W[in_pix, out_pix], packed per-window with
    # window-local row indexing (rows = S[w]..S[w]+127)
    W = np.zeros((576, 576), np.float32)
    for r in range(24):
        for c in range(24):
            o = r * 24 + c
            for dr in (-1, 0, 1):
                for dc in (-1, 0, 1):
                    rr, cc = r + dr, c + dc
                    if 0 <= rr < 24 and 0 <= cc < 24:
                        W[rr * 24 + cc, o] += wq[dr + 1, dc + 1]
    wcv = np.zeros((128, 576), np.float32)
    for (s, (o0, n)) in zip(S, ORANGE):
        wcv[:, o0:o0 + n] = W[s:s + 128, o0:o0 + n]

    Wdev = np.zeros((256, 10), np.float32)
    for i in range(12):
        for j in range(12):
            k = i * 12 + j
            r = k if k < 128 else k + 112
            Wdev[r, :] = wfq[:, (i + 1) * 14 + (j + 1)] / 128.0
    ident = np.eye(128, dtype=np.float16)
    return (wcv.astype(np.float16), Wdev.astype(np.float16), ident)


def _get_program():
    nc = _cache.get("prog")
    if nc is None:
        nc = _build()
        _cache["prog"] = nc
    return nc


def run(x, conv_w, fc_w, trace=False, **kw):
    from concourse.bass_utils import run_bass_kernel_spmd

    x2d = np.ascontiguousarray(np.asarray(x, np.float32).reshape(B, 576))
    wcv, Wdev, ident = _prep(conv_w, fc_w)
    nc = _get_program()
    in_maps = [{"x": np.ascontiguousarray(x2d[c * NPC:(c + 1) * NPC]),
                "wcv": wcv, "wfc": Wdev, "ident": ident}
               for c in range(NCORES)]
    res = run_bass_kernel_spmd(nc, in_maps,
                               core_ids=list(range(NCORES)),
                               trace=trace, **kw)
    out = np.concatenate([np.asarray(r["out"]).T for r in res.results], axis=0)
    return np.ascontiguousarray(out.astype(np.float32)), res


def kernel(x, conv_w, fc_w):
    out, _ = run(x, conv_w, fc_w, trace=False)
    return out

